# revision 7
# baseline (speedup 1.0000x reference)
"""GCN (2x GCNConv + mean-pool + FC) on 8 Trainium2 NeuronCores.

Sharding: nodes (and their incident in-edges) are partitioned contiguously
across 8 cores (dst-owner partitioning).  Each layer's propagate step is a
gather of source-node feature rows (dma_gather, int16 indices windowed over
4 table windows) followed by a segmented-sum implemented as one-hot matmuls
on the tensor engine.  The per-shard feature tables are replicated between
layers with an AllGather collective.  Pooling is a one-hot matmul over
graph ids (mod 128) + an AllGather + a host-prepared placement-matrix
matmul; the FC runs replicated on every core.
"""

import sys

sys.path.insert(0, "/opt/trn_rl_repo")

import numpy as np

# ---------------------------------------------------------------- constants
N = 100000
E = 1600000
G = 512
IN_C, H1, H2, OUT_C = 3, 64, 128, 2

NCORES = 8
SH = N // NCORES          # 12500 nodes per shard
NB = 98                   # 128-node tiles per shard (97*128+84)
SHP = NB * 128            # 12544 padded shard rows
WIN = 2 * SHP             # 25088 table rows per gather window
NW = 4                    # windows (4*WIN == 8*SHP)
TROW = 128                # padded bf16 table row (256B)
CHT = 8                   # tiles per gather chunk (1024 edges; >1024 idx overflows the SWDGE descriptor carveout)


# ---------------------------------------------------------------- host prep
def preprocess(edge_index, batch):
    """Build all per-core index metadata. Pure index manipulation."""
    src = np.asarray(edge_index[0], dtype=np.int64)
    dst = np.asarray(edge_index[1], dtype=np.int64)
    batch = np.asarray(batch, dtype=np.int64)

    owner = dst // SH
    # gather-table row for each source node (partition-major within shard)
    s_o = src // SH
    s_l = src - s_o * SH
    s_t = s_l // 128
    s_p = s_l - s_t * 128
    r_global = s_o * SHP + s_p * NB + s_t
    win = r_global // WIN
    idx16 = (r_global - win * WIN).astype(np.int16)

    dst_local = dst - owner * SH
    blk = dst_local // 128
    dstslot = (dst_local - blk * 128).astype(np.int16)

    # per (core, window, block) counts -> global tile counts
    key = ((owner * NW + win) * NB + blk).astype(np.int64)
    counts = np.bincount(key, minlength=NCORES * NW * NB).reshape(NCORES, NW, NB)
    t_wb = np.maximum(1, -(-counts.max(axis=0) // 128))  # [NW, NB]
    ntt_w = t_wb.sum(axis=1)                             # tiles per window
    ntt = int(ntt_w.sum())

    # tile -> block map per window (static across cores)
    tile_blk = [np.repeat(np.arange(NB), t_wb[w]) for w in range(NW)]

    # group start offsets (in tiles) within each window stream
    g_off = np.zeros((NW, NB), np.int64)
    for w in range(NW):
        g_off[w] = np.concatenate([[0], np.cumsum(t_wb[w])[:-1]])

    order = np.argsort(key, kind="stable")

    per_core = []
    for c in range(NCORES):
        idx_stream = np.zeros(ntt * 128, np.int16)
        slot_stream = np.full(ntt * 128, -1, np.int16)
        sel = order[(owner[order] == c)]
        ew = win[sel]
        eb = blk[sel]
        # position of each edge within its (w,b) group
        kcb = counts[c]  # [NW, NB]
        base_w = np.concatenate([[0], np.cumsum(ntt_w)[:-1]])
        within = np.zeros(len(sel), np.int64)
        pos0 = 0
        starts = {}
        # edges in `sel` are sorted by (w, b) already (stable sort by key)
        gkey = ew * NB + eb
        uniq, first = np.unique(gkey, return_index=True)
        for u, f in zip(uniq, first):
            w_, b_ = divmod(int(u), NB)
            cnt = int(kcb[w_, b_])
            pos = (base_w[w_] + g_off[w_, b_]) * 128
            s = slice(f, f + cnt)
            idx_stream[pos : pos + cnt] = idx16[sel[s]]
            slot_stream[pos : pos + cnt] = dstslot[sel[s]]
        # pad entries already idx 0 / slot -1

        nwrap = ntt * 8
        idx_img = np.zeros((128, nwrap), np.int16)
        w16 = idx_stream.reshape(nwrap, 16).T  # [16, nwrap]
        idx_img[:] = np.tile(w16, (8, 1))
        slot_img = slot_stream.reshape(ntt, 128).T.copy()  # [128, ntt]

        # rowptr of in-degree per local node, edge-image layout [128, NB]
        dl = dst_local[owner == c]
        dsort = np.sort(dl)
        rp = np.searchsorted(dsort, np.arange(SHP + 1))
        rp_lo = rp[:SHP].reshape(NB, 128).T.astype(np.int32).copy()
        rp_hi = rp[1 : SHP + 1].reshape(NB, 128).T.astype(np.int32).copy()

        # pooling: graph id mod 128 per node, edge-image layout, pad -1
        gl = np.full(SHP, -1, np.int64)
        gl[:SH] = batch[c * SH : (c + 1) * SH] % 128
        glocal_img = gl.reshape(NB, 128).T.astype(np.int16).copy()

        per_core.append(
            dict(idx_img=idx_img, slot_img=slot_img, rp_lo=rp_lo, rp_hi=rp_hi,
                 glocal_img=glocal_img)
        )

    # placement matrix M [8*128, 512]
    M = np.zeros((NCORES * 128, G), np.float32)
    for c in range(NCORES):
        for g in np.unique(batch[c * SH : (c + 1) * SH]):
            M[c * 128 + int(g) % 128, int(g)] = 1.0

    # per-window chunk tile counts (static)
    chunks = []
    for w in range(NW):
        n = int(ntt_w[w])
        ch = [CHT] * (n // CHT)
        if n % CHT:
            ch.append(n % CHT)
        chunks.append(ch)

    meta = dict(ntt=ntt, ntt_w=[int(x) for x in ntt_w], t_wb=t_wb,
                tile_blk=tile_blk, chunks=chunks, M=M)
    return meta, per_core


def _xt_img(x_shard):
    """x [SH,3] -> [3, SHP] f32 padded with zeros."""
    out = np.zeros((IN_C, SHP), np.float32)
    out[:, :SH] = x_shard.T
    return out


# ---------------------------------------------------------------- device kernel
def build_kernel(meta):
    from concourse import bass, bacc, tile, mybir
    f32 = mybir.dt.float32
    bf16 = mybir.dt.bfloat16
    i16 = mybir.dt.int16
    i32 = mybir.dt.int32

    ntt = meta["ntt"]
    ntt_w = meta["ntt_w"]
    t_wb = meta["t_wb"]
    chunks = meta["chunks"]

    nc = bacc.Bacc("TRN2", target_bir_lowering=False, debug=False,
                   num_devices=NCORES)

    # --- external inputs
    d_xt = nc.dram_tensor("xt", [IN_C, SHP], f32, kind="ExternalInput")
    d_w1 = nc.dram_tensor("w1", [IN_C, H1], f32, kind="ExternalInput")
    d_b1 = nc.dram_tensor("b1r", [1, H1], f32, kind="ExternalInput")
    d_w2 = nc.dram_tensor("w2", [H1, H2], f32, kind="ExternalInput")
    d_b2 = nc.dram_tensor("b2r", [1, H2], f32, kind="ExternalInput")
    d_wfc = nc.dram_tensor("wfc", [H2, OUT_C], f32, kind="ExternalInput")
    d_bfc = nc.dram_tensor("bfc2", [OUT_C, 1], f32, kind="ExternalInput")
    d_rplo = nc.dram_tensor("rp_lo", [128, NB], i32, kind="ExternalInput")
    d_rphi = nc.dram_tensor("rp_hi", [128, NB], i32, kind="ExternalInput")
    d_idx = nc.dram_tensor("idx_img", [128, ntt * 8], i16, kind="ExternalInput")
    d_slot = nc.dram_tensor("slot_img", [128, ntt], i16, kind="ExternalInput")
    d_gloc = nc.dram_tensor("glocal_img", [128, NB], i16, kind="ExternalInput")
    d_M = nc.dram_tensor("Mmat", [NCORES * 128, G], f32, kind="ExternalInput")
    d_ident = nc.dram_tensor("ident", [128, 128], f32, kind="ExternalInput")

    d_out = nc.dram_tensor("outT", [OUT_C, G], f32, kind="ExternalOutput")

    with tile.TileContext(nc) as tc:
        with (
            tc.tile_pool(name="static", bufs=1) as st,
            tc.tile_pool(name="gpool", bufs=3) as gp,
            tc.tile_pool(name="ipool", bufs=3) as ip,
            tc.tile_pool(name="spool", bufs=4) as sp,
            tc.tile_pool(name="tpool", bufs=3) as tp,
            tc.tile_pool(name="ps_seg", bufs=4, space="PSUM") as ps_seg,
            tc.tile_pool(name="ps_big", bufs=2, space="PSUM") as ps_big,
            tc.tile_pool(name="ps_pool", bufs=1, space="PSUM") as ps_pool,
            tc.tile_pool(name="dram", bufs=1, space="DRAM") as dram,
        ):
            # ---- static SBUF
            z1_loc = st.tile([128, NB, TROW], bf16)
            z2_loc = st.tile([128, NB, TROW], bf16)
            agg = st.tile([128, NB, H1], f32)
            dinv = st.tile([128, NB], f32)
            iota = st.tile([128, 128], i16)
            ident = st.tile([128, 128], f32)
            xt = st.tile([IN_C, SHP], f32)
            w1 = st.tile([IN_C, H1], f32)
            w2 = st.tile([H1, H2], f32)
            wfc = st.tile([H2, OUT_C], f32)
            bfc = st.tile([OUT_C, 1], f32)
            b1r = st.tile([1, H1], f32)
            b2r = st.tile([1, H2], f32)
            b1bc = st.tile([128, H1], f32)
            b2bc = st.tile([128, H2], f32)
            ones1 = st.tile([1, 128], f32)
            onecol = st.tile([128, 1], bf16)
            rplo = st.tile([128, NB], i32)
            rphi = st.tile([128, NB], i32)
            gloc = st.tile([128, NB], i16)
            degi = st.tile([128, NB], i32)
            degf = st.tile([128, NB], f32)

            # ---- internal DRAM
            z1_sh = dram.tile([SHP, TROW], bf16)
            z1_full = dram.tile([NCORES * SHP, TROW], bf16)
            z2_sh = dram.tile([SHP, TROW], bf16)
            z2_full = dram.tile([NCORES * SHP, TROW], bf16)
            pool_sh = dram.tile([128, H2 + 4], f32)
            pool_ag = dram.tile([NCORES * 128, H2 + 4], f32)

            # ---- phase 0: constants / dinv / z1
            nc.sync.dma_start(xt[:], d_xt.ap())
            nc.sync.dma_start(w1[:], d_w1.ap())
            nc.sync.dma_start(w2[:], d_w2.ap())
            nc.sync.dma_start(wfc[:], d_wfc.ap())
            nc.sync.dma_start(bfc[:], d_bfc.ap())
            nc.sync.dma_start(b1r[:], d_b1.ap())
            nc.sync.dma_start(b2r[:], d_b2.ap())
            nc.sync.dma_start(rplo[:], d_rplo.ap())
            nc.sync.dma_start(rphi[:], d_rphi.ap())
            nc.sync.dma_start(gloc[:], d_gloc.ap())
            nc.sync.dma_start(ident[:], d_ident.ap())
            nc.gpsimd.iota(iota[:], pattern=[[1, 128]], base=0,
                           channel_multiplier=0)
            nc.vector.memset(ones1[:], 1.0)
            nc.vector.memset(onecol[:], 1.0)
            nc.vector.memset(z1_loc[:, :, H1:], 0.0)
            nc.vector.memset(z2_loc[:, :, H1:], 0.0)

            # bias broadcast rows -> [128, F]
            pb = ps_big.tile([128, H1], f32, tag="big")
            nc.tensor.matmul(pb[:], ones1[:], b1r[:], start=True, stop=True)
            nc.vector.tensor_copy(b1bc[:], pb[:])
            pb2 = ps_big.tile([128, H2], f32, tag="big")
            nc.tensor.matmul(pb2[:], ones1[:], b2r[:], start=True, stop=True)
            nc.vector.tensor_copy(b2bc[:], pb2[:])

            # dinv = rsqrt(rp_hi - rp_lo + 1)
            nc.vector.tensor_tensor(degi[:], rphi[:], rplo[:],
                                    mybir.AluOpType.subtract)
            nc.vector.tensor_copy(degf[:], degi[:])
            nc.vector.tensor_scalar_add(degf[:], degf[:], 1.0)
            nc.scalar.activation(degf[:], degf[:],
                                 mybir.ActivationFunctionType.Sqrt)
            nc.vector.reciprocal(dinv[:], degf[:])

            # z1 = dinv * (x @ W1), tile by tile
            for t in range(NB):
                pw = ps_seg.tile([128, H1], f32, tag="pg")
                nc.tensor.matmul(pw[:], xt[:, t * 128:(t + 1) * 128], w1[:],
                                 start=True, stop=True)
                nc.vector.tensor_tensor(
                    z1_loc[:, t, :H1], pw[:],
                    dinv[:, t:t + 1].broadcast_to([128, H1]),
                    mybir.AluOpType.mult)
            nc.sync.dma_start(
                z1_sh[:].rearrange("(p t) f -> p t f", p=128), z1_loc[:])
            nc.gpsimd.collective_compute(
                "AllGather", mybir.AluOpType.bypass,
                replica_groups=[list(range(NCORES))],
                ins=[z1_sh.opt()], outs=[z1_full.opt()])

            # ---- segmented-sum layer
            def seg_layer(z_full):
                tbase = 0
                for w in range(NW):
                    # gather chunks for this window
                    gtiles = []
                    ti0 = 0
                    for nt in chunks[w]:
                        g_t = gp.tile([128, CHT, TROW], bf16, tag="gt")
                        idx_t = ip.tile([128, CHT * 8], i16, tag="ix")
                        c0 = (tbase + ti0) * 8
                        nc.sync.dma_start(idx_t[:, :nt * 8],
                                          d_idx.ap()[:, c0:c0 + nt * 8])
                        nc.gpsimd.dma_gather(
                            g_t[:, :nt, :],
                            z_full[w * WIN:(w + 1) * WIN, :],
                            idx_t[:, :nt * 8],
                            nt * 128, nt * 128, TROW)
                        gtiles.append((g_t, ti0, nt))
                        ti0 += nt

                    slot_t = ip.tile([128, ntt_w[w]], i16, tag="sl")
                    nc.sync.dma_start(slot_t[:],
                                      d_slot.ap()[:, tbase:tbase + ntt_w[w]])

                    ci = 0
                    ti = 0  # tile index within window
                    for b in range(NB):
                        T = int(t_wb[w][b])
                        pg = ps_seg.tile([128, H1], f32, tag="pg")
                        for k in range(T):
                            g_t, ti0, nt = gtiles[ci]
                            if ti - ti0 >= nt:
                                ci += 1
                                g_t, ti0, nt = gtiles[ci]
                            s_t = sp.tile([128, 128], bf16, tag="s")
                            nc.vector.tensor_tensor(
                                s_t[:],
                                slot_t[:, ti:ti + 1].broadcast_to([128, 128]),
                                iota[:], mybir.AluOpType.is_equal)
                            nc.tensor.matmul(
                                pg[:], s_t[:], g_t[:, ti - ti0, :H1],
                                start=(k == 0), stop=(k == T - 1))
                            ti += 1
                        if w == 0:
                            nc.vector.tensor_copy(agg[:, b, :], pg[:])
                        else:
                            nc.vector.tensor_tensor(agg[:, b, :], pg[:],
                                                    agg[:, b, :],
                                                    mybir.AluOpType.add)
                    tbase += ntt_w[w]

            # ---- layer 1
            seg_layer(z1_full)
            for t in range(NB):
                tmp = tp.tile([128, H1], f32, tag="tmp")
                dv = dinv[:, t:t + 1].broadcast_to([128, H1])
                nc.vector.tensor_tensor(tmp[:], agg[:, t, :], z1_loc[:, t, :H1],
                                        mybir.AluOpType.add)
                nc.vector.tensor_tensor(tmp[:], tmp[:], dv,
                                        mybir.AluOpType.mult)
                nc.vector.tensor_tensor(tmp[:], tmp[:], b1bc[:],
                                        mybir.AluOpType.add)
                nc.vector.tensor_scalar_max(tmp[:], tmp[:], 0.0)  # h1
                nc.vector.tensor_tensor(z2_loc[:, t, :H1], tmp[:], dv,
                                        mybir.AluOpType.mult)
            nc.sync.dma_start(
                z2_sh[:].rearrange("(p t) f -> p t f", p=128), z2_loc[:])
            nc.gpsimd.collective_compute(
                "AllGather", mybir.AluOpType.bypass,
                replica_groups=[list(range(NCORES))],
                ins=[z2_sh.opt()], outs=[z2_full.opt()])

            # ---- layer 2
            seg_layer(z2_full)
            p_pool = ps_pool.tile([128, H2], f32, tag="plh")
            p_cnt = ps_pool.tile([128, 4], f32, tag="plc")
            for t in range(NB):
                tmp = tp.tile([128, H1], f32, tag="tmp")
                dv = dinv[:, t:t + 1].broadcast_to([128, H1])
                nc.vector.tensor_tensor(tmp[:], agg[:, t, :], z2_loc[:, t, :H1],
                                        mybir.AluOpType.add)
                nc.vector.tensor_tensor(tmp[:], tmp[:], dv,
                                        mybir.AluOpType.mult)  # pre2
                ptr = ps_big.tile([H1, 128], f32, tag="big")
                nc.tensor.transpose(ptr[:], tmp[:], ident[:])
                pre2T = tp.tile([H1, 128], f32, tag="p2t")
                nc.vector.tensor_copy(pre2T[:], ptr[:])
                ph = ps_big.tile([128, H2], f32, tag="big")
                nc.tensor.matmul(ph[:], pre2T[:], w2[:], start=True, stop=True)
                h2 = tp.tile([128, H2], bf16, tag="h2")
                nc.vector.tensor_tensor(ph[:], ph[:], b2bc[:],
                                        mybir.AluOpType.add)
                nc.vector.tensor_scalar_max(h2[:], ph[:], 0.0)
                s_t = sp.tile([128, 128], bf16, tag="s")
                nc.vector.tensor_tensor(
                    s_t[:], gloc[:, t:t + 1].broadcast_to([128, 128]),
                    iota[:], mybir.AluOpType.is_equal)
                nc.tensor.matmul(p_pool[:], s_t[:], h2[:],
                                 start=(t == 0), stop=(t == NB - 1))
                nc.tensor.matmul(p_cnt[:, 0:1], s_t[:], onecol[:],
                                 start=(t == 0), stop=(t == NB - 1))

            # ---- pooling combine + FC
            pool_sb = st.tile([128, H2 + 4], f32)
            nc.vector.memset(pool_sb[:, H2 + 1:], 0.0)
            nc.vector.tensor_copy(pool_sb[:, :H2], p_pool[:])
            nc.vector.tensor_copy(pool_sb[:, H2:H2 + 1], p_cnt[:, 0:1])
            nc.gpsimd.dma_start(pool_sh[:], pool_sb[:])
            nc.gpsimd.collective_compute(
                "AllGather", mybir.AluOpType.bypass,
                replica_groups=[list(range(NCORES))],
                ins=[pool_sh.opt()], outs=[pool_ag.opt()])

            agp = st.tile([128, NCORES, H2 + 4], f32)
            nc.sync.dma_start(
                agp[:], pool_ag[:].rearrange("(c p) f -> p c f", c=NCORES))
            meanT = st.tile([128, G], f32)
            GB = G // 128
            for gb in range(GB):
                pf = ps_big.tile([128, H2 + 4], f32, tag="big")
                for c in range(NCORES):
                    mt = tp.tile([128, 128], f32, tag="mt")
                    nc.sync.dma_start(
                        mt[:],
                        d_M.ap()[c * 128:(c + 1) * 128,
                                 gb * 128:(gb + 1) * 128])
                    nc.tensor.matmul(pf[:, :H2 + 1], mt[:],
                                     agp[:, c, :H2 + 1],
                                     start=(c == 0), stop=(c == NCORES - 1))
                cnt = tp.tile([128, 1], f32, tag="cnt")
                nc.vector.tensor_scalar_max(cnt[:], pf[:, H2:H2 + 1], 1.0)
                rec = tp.tile([128, 1], f32, tag="rec")
                nc.vector.reciprocal(rec[:], cnt[:])
                mean = tp.tile([128, H2], f32, tag="mean")
                nc.vector.tensor_tensor(mean[:], pf[:, :H2],
                                        rec[:].broadcast_to([128, H2]),
                                        mybir.AluOpType.mult)
                ptm = ps_big.tile([128, 128], f32, tag="big")
                nc.tensor.transpose(ptm[:], mean[:], ident[:])
                nc.vector.tensor_copy(meanT[:, gb * 128:(gb + 1) * 128],
                                      ptm[:])
            pfc = ps_big.tile([OUT_C, G], f32, tag="big")
            nc.tensor.matmul(pfc[:], wfc[:], meanT[:], start=True, stop=True)
            outsb = st.tile([OUT_C, G], f32)
            nc.vector.tensor_tensor(outsb[:], pfc[:],
                                    bfc[:].broadcast_to([OUT_C, G]),
                                    mybir.AluOpType.add)
            nc.sync.dma_start(d_out.ap(), outsb[:])

    nc.compile()
    return nc


_CACHE = {}


def _run(inputs, trace=False):
    from concourse.bass_utils import run_bass_kernel_spmd

    edge_index = np.asarray(inputs["edge_index"])
    batch = np.asarray(inputs["batch"])
    key = "k"
    if key not in _CACHE:
        meta, per_core = preprocess(edge_index, batch)
        nc = build_kernel(meta)
        _CACHE[key] = (meta, per_core, nc)
    meta, per_core, nc = _CACHE[key]

    x = np.asarray(inputs["x"], np.float32)
    W1 = np.asarray(inputs["W1"], np.float32)
    b1 = np.asarray(inputs["b1"], np.float32).reshape(1, H1)
    W2 = np.asarray(inputs["W2"], np.float32)
    b2 = np.asarray(inputs["b2"], np.float32).reshape(1, H2)
    Wfc = np.asarray(inputs["Wfc"], np.float32)
    bfc = np.asarray(inputs["bfc"], np.float32).reshape(OUT_C, 1)
    ident = np.eye(128, dtype=np.float32)

    in_maps = []
    for c in range(NCORES):
        pc = per_core[c]
        in_maps.append({
            "xt": _xt_img(x[c * SH:(c + 1) * SH]),
            "w1": W1, "b1r": b1, "w2": W2, "b2r": b2,
            "wfc": Wfc, "bfc2": bfc,
            "rp_lo": pc["rp_lo"], "rp_hi": pc["rp_hi"],
            "idx_img": pc["idx_img"], "slot_img": pc["slot_img"],
            "glocal_img": pc["glocal_img"],
            "Mmat": meta["M"], "ident": ident,
        })
    res = run_bass_kernel_spmd(nc, in_maps, list(range(NCORES)), trace=trace)
    out = res.results[0]["outT"].T.copy()  # [G, 2]
    return out.astype(np.float32), res


def kernel(**inputs):
    out, _ = _run(inputs)
    return out


# numpy simulation of the device algorithm (for validation)
def numpy_sim(inputs, meta, per_core, use_bf16=True):
    import ml_dtypes

    bf16 = ml_dtypes.bfloat16

    def q(a):
        return a.astype(bf16).astype(np.float32) if use_bf16 else a

    x = np.asarray(inputs["x"], np.float32)
    W1 = np.asarray(inputs["W1"], np.float32)
    b1 = np.asarray(inputs["b1"], np.float32)
    W2 = np.asarray(inputs["W2"], np.float32)
    b2 = np.asarray(inputs["b2"], np.float32)
    Wfc = np.asarray(inputs["Wfc"], np.float32)
    bfc = np.asarray(inputs["bfc"], np.float32)

    ntt = meta["ntt"]
    tile_blk = meta["tile_blk"]
    ntt_w = meta["ntt_w"]

    # per-core dinv
    dinv = []
    for c in range(NCORES):
        pc = per_core[c]
        deg = (pc["rp_hi"].astype(np.int64) - pc["rp_lo"].astype(np.int64)) + 1
        dinv.append(1.0 / np.sqrt(deg.astype(np.float32)))  # [128, NB]

    def seg_layer(z_full_q, core):
        """z_full_q: [8*SHP, TROW] quantized table; returns agg [128, NB, 64]."""
        pc = per_core[core]
        agg = np.zeros((128, NB, H1), np.float32)
        tbase = 0
        for w in range(NW):
            for ti in range(ntt_w[w]):
                t = tbase + ti
                b = int(tile_blk[w][ti])
                idxs = pc["idx_img"][:16, t * 8 : (t + 1) * 8].T.reshape(-1)  # 128
                rows = z_full_q[w * WIN + idxs.astype(np.int64), :H1]  # [128, 64]
                slots = pc["slot_img"][:, t].astype(np.int64)  # [128]
                S = np.zeros((128, 128), np.float32)
                val = slots >= 0
                S[np.arange(128)[val], slots[val]] = 1.0
                agg[:, b, :] += S.T @ rows
            tbase += ntt_w[w]
        return agg

    # layer 1 tables
    z1_full = np.zeros((NCORES * SHP, TROW), np.float32)
    xw1_all = []
    for c in range(NCORES):
        xt = _xt_img(x[c * SH : (c + 1) * SH])  # [3, SHP]
        xw1 = (xt.T @ W1)  # [SHP, 64]
        xw1_img = xw1.reshape(NB, 128, H1).transpose(1, 0, 2)  # [128, NB, 64]
        z1 = xw1_img * dinv[c][:, :, None]
        # table rows partition-major: row p*NB+t
        z1_full[c * SHP : (c + 1) * SHP, :H1] = q(z1).transpose(0, 1, 2).reshape(
            128 * NB, H1
        )
        xw1_all.append(xw1_img)
    z1q = q(z1_full)

    h1_all, z2_all = [], []
    z2_full = np.zeros((NCORES * SHP, TROW), np.float32)
    for c in range(NCORES):
        agg1 = seg_layer(z1q, c)
        z1_loc = z1q[c * SHP : (c + 1) * SHP, :H1].reshape(128, NB, H1)
        h1 = np.maximum(dinv[c][:, :, None] * (agg1 + z1_loc) + b1, 0.0)
        z2 = h1 * dinv[c][:, :, None]
        z2_full[c * SHP : (c + 1) * SHP, :H1] = q(z2).reshape(128 * NB, H1)
        h1_all.append(h1)
        z2_all.append(z2)
    z2q = q(z2_full)

    pool_part = np.zeros((NCORES, 128, H2 + 1), np.float32)
    for c in range(NCORES):
        agg2 = seg_layer(z2q, c)
        z2_loc = z2q[c * SHP : (c + 1) * SHP, :H1].reshape(128, NB, H1)
        pre2 = dinv[c][:, :, None] * (agg2 + z2_loc)
        h2 = np.maximum(pre2 @ W2 + b2, 0.0)  # [128, NB, 128]
        h2q = q(h2)
        pc = per_core[c]
        for t in range(NB):
            slots = pc["glocal_img"][:, t].astype(np.int64)
            Sp = np.zeros((128, 128), np.float32)
            val = slots >= 0
            Sp[np.arange(128)[val], slots[val]] = 1.0
            pool_part[c, :, :H2] += Sp.T @ h2q[:, t, :]
            pool_part[c, :, H2] += Sp.sum(axis=0)

    ag = pool_part.reshape(NCORES * 128, H2 + 1)
    full = meta["M"].T @ ag  # [512, 129]
    cnt = np.maximum(full[:, H2], 1.0)
    mean = full[:, :H2] / cnt[:, None]
    return mean @ Wfc + bfc


# revision 8
# speedup vs baseline: 1.0178x; 1.0178x over previous
"""GCN (2x GCNConv + mean-pool + FC) on 8 Trainium2 NeuronCores.

Sharding: nodes (and their incident in-edges) are partitioned contiguously
across 8 cores (dst-owner partitioning).  Each layer's propagate step is a
gather of source-node feature rows (dma_gather, int16 indices windowed over
4 table windows) followed by a segmented-sum implemented as one-hot matmuls
on the tensor engine.  The per-shard feature tables are replicated between
layers with an AllGather collective.  Pooling is a one-hot matmul over
graph ids (mod 128) + an AllGather + a host-prepared placement-matrix
matmul; the FC runs replicated on every core.
"""

import sys

sys.path.insert(0, "/opt/trn_rl_repo")

import numpy as np

# ---------------------------------------------------------------- constants
N = 100000
E = 1600000
G = 512
IN_C, H1, H2, OUT_C = 3, 64, 128, 2

NCORES = 8
SH = N // NCORES          # 12500 nodes per shard
NB = 98                   # 128-node tiles per shard (97*128+84)
SHP = NB * 128            # 12544 padded shard rows
WIN = 2 * SHP             # 25088 table rows per gather window
NW = 4                    # windows (4*WIN == 8*SHP)
TROW = 128                # padded bf16 table row (256B)
CHT = 8                   # tiles per gather chunk (1024 edges; >1024 idx overflows the SWDGE descriptor carveout)


# ---------------------------------------------------------------- host prep
def preprocess(edge_index, batch):
    """Build all per-core index metadata. Pure index manipulation."""
    src = np.asarray(edge_index[0], dtype=np.int64)
    dst = np.asarray(edge_index[1], dtype=np.int64)
    batch = np.asarray(batch, dtype=np.int64)

    owner = dst // SH
    # gather-table row for each source node (partition-major within shard)
    s_o = src // SH
    s_l = src - s_o * SH
    s_t = s_l // 128
    s_p = s_l - s_t * 128
    r_global = s_o * SHP + s_p * NB + s_t
    win = r_global // WIN
    idx16 = (r_global - win * WIN).astype(np.int16)

    dst_local = dst - owner * SH
    blk = dst_local // 128
    dstslot = (dst_local - blk * 128).astype(np.int16)

    # per (core, window, block) counts -> global tile counts
    key = ((owner * NW + win) * NB + blk).astype(np.int64)
    counts = np.bincount(key, minlength=NCORES * NW * NB).reshape(NCORES, NW, NB)
    t_wb = np.maximum(1, -(-counts.max(axis=0) // 128))  # [NW, NB]
    ntt_w = t_wb.sum(axis=1)                             # tiles per window
    ntt = int(ntt_w.sum())

    # tile -> block map per window (static across cores)
    tile_blk = [np.repeat(np.arange(NB), t_wb[w]) for w in range(NW)]

    # group start offsets (in tiles) within each window stream
    g_off = np.zeros((NW, NB), np.int64)
    for w in range(NW):
        g_off[w] = np.concatenate([[0], np.cumsum(t_wb[w])[:-1]])

    order = np.argsort(key, kind="stable")

    per_core = []
    for c in range(NCORES):
        idx_stream = np.zeros(ntt * 128, np.int16)
        slot_stream = np.full(ntt * 128, -1, np.int16)
        sel = order[(owner[order] == c)]
        ew = win[sel]
        eb = blk[sel]
        # position of each edge within its (w,b) group
        kcb = counts[c]  # [NW, NB]
        base_w = np.concatenate([[0], np.cumsum(ntt_w)[:-1]])
        within = np.zeros(len(sel), np.int64)
        pos0 = 0
        starts = {}
        # edges in `sel` are sorted by (w, b) already (stable sort by key)
        gkey = ew * NB + eb
        uniq, first = np.unique(gkey, return_index=True)
        for u, f in zip(uniq, first):
            w_, b_ = divmod(int(u), NB)
            cnt = int(kcb[w_, b_])
            pos = (base_w[w_] + g_off[w_, b_]) * 128
            s = slice(f, f + cnt)
            idx_stream[pos : pos + cnt] = idx16[sel[s]]
            slot_stream[pos : pos + cnt] = dstslot[sel[s]]
        # pad entries already idx 0 / slot -1

        nwrap = ntt * 8
        idx_img = np.zeros((128, nwrap), np.int16)
        w16 = idx_stream.reshape(nwrap, 16).T  # [16, nwrap]
        idx_img[:] = np.tile(w16, (8, 1))
        slot_img = slot_stream.reshape(ntt, 128).T.copy()  # [128, ntt]

        # rowptr of in-degree per local node, edge-image layout [128, NB]
        dl = dst_local[owner == c]
        dsort = np.sort(dl)
        rp = np.searchsorted(dsort, np.arange(SHP + 1))
        rp_lo = rp[:SHP].reshape(NB, 128).T.astype(np.int32).copy()
        rp_hi = rp[1 : SHP + 1].reshape(NB, 128).T.astype(np.int32).copy()

        # pooling: graph id mod 128 per node, edge-image layout, pad -1
        gl = np.full(SHP, -1, np.int64)
        gl[:SH] = batch[c * SH : (c + 1) * SH] % 128
        glocal_img = gl.reshape(NB, 128).T.astype(np.int16).copy()

        per_core.append(
            dict(idx_img=idx_img, slot_img=slot_img, rp_lo=rp_lo, rp_hi=rp_hi,
                 glocal_img=glocal_img)
        )

    # placement matrix M [8*128, 512]
    M = np.zeros((NCORES * 128, G), np.float32)
    for c in range(NCORES):
        for g in np.unique(batch[c * SH : (c + 1) * SH]):
            M[c * 128 + int(g) % 128, int(g)] = 1.0

    # per-window chunk tile counts (static)
    chunks = []
    for w in range(NW):
        n = int(ntt_w[w])
        ch = [CHT] * (n // CHT)
        if n % CHT:
            ch.append(n % CHT)
        chunks.append(ch)

    meta = dict(ntt=ntt, ntt_w=[int(x) for x in ntt_w], t_wb=t_wb,
                tile_blk=tile_blk, chunks=chunks, M=M)
    return meta, per_core


def _xt_img(x_shard):
    """x [SH,3] -> [3, SHP] f32 padded with zeros."""
    out = np.zeros((IN_C, SHP), np.float32)
    out[:, :SH] = x_shard.T
    return out


# ---------------------------------------------------------------- device kernel
def build_kernel(meta):
    from concourse import bass, bacc, tile, mybir
    f32 = mybir.dt.float32
    bf16 = mybir.dt.bfloat16
    i16 = mybir.dt.int16
    i32 = mybir.dt.int32

    ntt = meta["ntt"]
    ntt_w = meta["ntt_w"]
    t_wb = meta["t_wb"]
    chunks = meta["chunks"]

    nc = bacc.Bacc("TRN2", target_bir_lowering=False, debug=False,
                   num_devices=NCORES, num_swdge_queues=4)

    # --- external inputs
    d_xt = nc.dram_tensor("xt", [IN_C, SHP], f32, kind="ExternalInput")
    d_w1 = nc.dram_tensor("w1", [IN_C, H1], f32, kind="ExternalInput")
    d_b1 = nc.dram_tensor("b1r", [1, H1], f32, kind="ExternalInput")
    d_w2 = nc.dram_tensor("w2", [H1, H2], f32, kind="ExternalInput")
    d_b2 = nc.dram_tensor("b2r", [1, H2], f32, kind="ExternalInput")
    d_wfc = nc.dram_tensor("wfc", [H2, OUT_C], f32, kind="ExternalInput")
    d_bfc = nc.dram_tensor("bfc2", [OUT_C, 1], f32, kind="ExternalInput")
    d_rplo = nc.dram_tensor("rp_lo", [128, NB], i32, kind="ExternalInput")
    d_rphi = nc.dram_tensor("rp_hi", [128, NB], i32, kind="ExternalInput")
    d_idx = nc.dram_tensor("idx_img", [128, ntt * 8], i16, kind="ExternalInput")
    d_slot = nc.dram_tensor("slot_img", [128, ntt], i16, kind="ExternalInput")
    d_gloc = nc.dram_tensor("glocal_img", [128, NB], i16, kind="ExternalInput")
    d_M = nc.dram_tensor("Mmat", [NCORES * 128, G], f32, kind="ExternalInput")
    d_ident = nc.dram_tensor("ident", [128, 128], f32, kind="ExternalInput")

    d_out = nc.dram_tensor("outT", [OUT_C, G], f32, kind="ExternalOutput")

    with tile.TileContext(nc) as tc:
        with (
            tc.tile_pool(name="static", bufs=1) as st,
            tc.tile_pool(name="gpool", bufs=3) as gp,
            tc.tile_pool(name="ipool", bufs=3) as ip,
            tc.tile_pool(name="spool", bufs=4) as sp,
            tc.tile_pool(name="tpool", bufs=3) as tp,
            tc.tile_pool(name="ps_seg", bufs=4, space="PSUM") as ps_seg,
            tc.tile_pool(name="ps_big", bufs=2, space="PSUM") as ps_big,
            tc.tile_pool(name="ps_pool", bufs=1, space="PSUM") as ps_pool,
            tc.tile_pool(name="dram", bufs=1, space="DRAM") as dram,
        ):
            # ---- static SBUF
            z1_loc = st.tile([128, NB, TROW], bf16)
            z2_loc = st.tile([128, NB, TROW], bf16)
            agg = st.tile([128, NB, H1], f32)
            dinv = st.tile([128, NB], f32)
            iota = st.tile([128, 128], i16)
            ident = st.tile([128, 128], f32)
            xt = st.tile([IN_C, SHP], f32)
            w1 = st.tile([IN_C, H1], f32)
            w2 = st.tile([H1, H2], f32)
            wfc = st.tile([H2, OUT_C], f32)
            bfc = st.tile([OUT_C, 1], f32)
            b1r = st.tile([1, H1], f32)
            b2r = st.tile([1, H2], f32)
            b1bc = st.tile([128, H1], f32)
            b2bc = st.tile([128, H2], f32)
            ones1 = st.tile([1, 128], f32)
            onecol = st.tile([128, 1], bf16)
            rplo = st.tile([128, NB], i32)
            rphi = st.tile([128, NB], i32)
            gloc = st.tile([128, NB], i16)
            degi = st.tile([128, NB], i32)
            degf = st.tile([128, NB], f32)

            # ---- internal DRAM
            z1_sh = dram.tile([SHP, TROW], bf16)
            z1_full = dram.tile([NCORES * SHP, TROW], bf16)
            z2_sh = dram.tile([SHP, TROW], bf16)
            z2_full = dram.tile([NCORES * SHP, TROW], bf16)
            pool_sh = dram.tile([128, H2 + 4], f32)
            pool_ag = dram.tile([NCORES * 128, H2 + 4], f32)

            # ---- phase 0: constants / dinv / z1
            nc.sync.dma_start(xt[:], d_xt.ap())
            nc.sync.dma_start(w1[:], d_w1.ap())
            nc.sync.dma_start(w2[:], d_w2.ap())
            nc.sync.dma_start(wfc[:], d_wfc.ap())
            nc.sync.dma_start(bfc[:], d_bfc.ap())
            nc.sync.dma_start(b1r[:], d_b1.ap())
            nc.sync.dma_start(b2r[:], d_b2.ap())
            nc.sync.dma_start(rplo[:], d_rplo.ap())
            nc.sync.dma_start(rphi[:], d_rphi.ap())
            nc.sync.dma_start(gloc[:], d_gloc.ap())
            nc.sync.dma_start(ident[:], d_ident.ap())
            nc.gpsimd.iota(iota[:], pattern=[[1, 128]], base=0,
                           channel_multiplier=0)
            nc.vector.memset(ones1[:], 1.0)
            nc.vector.memset(onecol[:], 1.0)
            nc.vector.memset(z1_loc[:, :, H1:], 0.0)
            nc.vector.memset(z2_loc[:, :, H1:], 0.0)

            # bias broadcast rows -> [128, F]
            pb = ps_big.tile([128, H1], f32, tag="big")
            nc.tensor.matmul(pb[:], ones1[:], b1r[:], start=True, stop=True)
            nc.vector.tensor_copy(b1bc[:], pb[:])
            pb2 = ps_big.tile([128, H2], f32, tag="big")
            nc.tensor.matmul(pb2[:], ones1[:], b2r[:], start=True, stop=True)
            nc.vector.tensor_copy(b2bc[:], pb2[:])

            # dinv = rsqrt(rp_hi - rp_lo + 1)
            nc.vector.tensor_tensor(degi[:], rphi[:], rplo[:],
                                    mybir.AluOpType.subtract)
            nc.vector.tensor_copy(degf[:], degi[:])
            nc.vector.tensor_scalar_add(degf[:], degf[:], 1.0)
            nc.scalar.activation(degf[:], degf[:],
                                 mybir.ActivationFunctionType.Sqrt)
            nc.vector.reciprocal(dinv[:], degf[:])

            # z1 = dinv * (x @ W1), tile by tile
            for t in range(NB):
                pw = ps_seg.tile([128, H1], f32, tag="pg")
                nc.tensor.matmul(pw[:], xt[:, t * 128:(t + 1) * 128], w1[:],
                                 start=True, stop=True)
                nc.vector.tensor_tensor(
                    z1_loc[:, t, :H1], pw[:],
                    dinv[:, t:t + 1].broadcast_to([128, H1]),
                    mybir.AluOpType.mult)
            nc.sync.dma_start(
                z1_sh[:].rearrange("(p t) f -> p t f", p=128), z1_loc[:])
            nc.gpsimd.collective_compute(
                "AllGather", mybir.AluOpType.bypass,
                replica_groups=[list(range(NCORES))],
                ins=[z1_sh.opt()], outs=[z1_full.opt()])

            # ---- segmented-sum layer
            def seg_layer(z_full):
                tbase = 0
                for w in range(NW):
                    # gather chunks for this window
                    gtiles = []
                    ti0 = 0
                    for nt in chunks[w]:
                        g_t = gp.tile([128, CHT, TROW], bf16, tag="gt")
                        idx_t = ip.tile([128, CHT * 8], i16, tag="ix")
                        c0 = (tbase + ti0) * 8
                        nc.sync.dma_start(idx_t[:, :nt * 8],
                                          d_idx.ap()[:, c0:c0 + nt * 8])
                        nc.gpsimd.dma_gather(
                            g_t[:, :nt, :],
                            z_full[w * WIN:(w + 1) * WIN, :],
                            idx_t[:, :nt * 8],
                            nt * 128, nt * 128, TROW, queue_num=w)
                        gtiles.append((g_t, ti0, nt))
                        ti0 += nt

                    slot_t = ip.tile([128, ntt_w[w]], i16, tag="sl")
                    nc.sync.dma_start(slot_t[:],
                                      d_slot.ap()[:, tbase:tbase + ntt_w[w]])

                    ci = 0
                    ti = 0  # tile index within window
                    for b in range(NB):
                        T = int(t_wb[w][b])
                        pg = ps_seg.tile([128, H1], f32, tag="pg")
                        for k in range(T):
                            g_t, ti0, nt = gtiles[ci]
                            if ti - ti0 >= nt:
                                ci += 1
                                g_t, ti0, nt = gtiles[ci]
                            s_t = sp.tile([128, 128], bf16, tag="s")
                            nc.vector.tensor_tensor(
                                s_t[:],
                                slot_t[:, ti:ti + 1].broadcast_to([128, 128]),
                                iota[:], mybir.AluOpType.is_equal)
                            nc.tensor.matmul(
                                pg[:], s_t[:], g_t[:, ti - ti0, :H1],
                                start=(k == 0), stop=(k == T - 1))
                            ti += 1
                        if w == 0:
                            nc.vector.tensor_copy(agg[:, b, :], pg[:])
                        else:
                            nc.vector.tensor_tensor(agg[:, b, :], pg[:],
                                                    agg[:, b, :],
                                                    mybir.AluOpType.add)
                    tbase += ntt_w[w]

            # ---- layer 1
            seg_layer(z1_full)
            for t in range(NB):
                tmp = tp.tile([128, H1], f32, tag="tmp")
                dv = dinv[:, t:t + 1].broadcast_to([128, H1])
                nc.vector.tensor_tensor(tmp[:], agg[:, t, :], z1_loc[:, t, :H1],
                                        mybir.AluOpType.add)
                nc.vector.tensor_tensor(tmp[:], tmp[:], dv,
                                        mybir.AluOpType.mult)
                nc.vector.tensor_tensor(tmp[:], tmp[:], b1bc[:],
                                        mybir.AluOpType.add)
                nc.vector.tensor_scalar_max(tmp[:], tmp[:], 0.0)  # h1
                nc.vector.tensor_tensor(z2_loc[:, t, :H1], tmp[:], dv,
                                        mybir.AluOpType.mult)
            nc.sync.dma_start(
                z2_sh[:].rearrange("(p t) f -> p t f", p=128), z2_loc[:])
            nc.gpsimd.collective_compute(
                "AllGather", mybir.AluOpType.bypass,
                replica_groups=[list(range(NCORES))],
                ins=[z2_sh.opt()], outs=[z2_full.opt()])

            # ---- layer 2
            seg_layer(z2_full)
            p_pool = ps_pool.tile([128, H2], f32, tag="plh")
            p_cnt = ps_pool.tile([128, 4], f32, tag="plc")
            for t in range(NB):
                tmp = tp.tile([128, H1], f32, tag="tmp")
                dv = dinv[:, t:t + 1].broadcast_to([128, H1])
                nc.vector.tensor_tensor(tmp[:], agg[:, t, :], z2_loc[:, t, :H1],
                                        mybir.AluOpType.add)
                nc.vector.tensor_tensor(tmp[:], tmp[:], dv,
                                        mybir.AluOpType.mult)  # pre2
                ptr = ps_big.tile([H1, 128], f32, tag="big")
                nc.tensor.transpose(ptr[:], tmp[:], ident[:])
                pre2T = tp.tile([H1, 128], f32, tag="p2t")
                nc.vector.tensor_copy(pre2T[:], ptr[:])
                ph = ps_big.tile([128, H2], f32, tag="big")
                nc.tensor.matmul(ph[:], pre2T[:], w2[:], start=True, stop=True)
                h2 = tp.tile([128, H2], bf16, tag="h2")
                nc.vector.tensor_tensor(ph[:], ph[:], b2bc[:],
                                        mybir.AluOpType.add)
                nc.vector.tensor_scalar_max(h2[:], ph[:], 0.0)
                s_t = sp.tile([128, 128], bf16, tag="s")
                nc.vector.tensor_tensor(
                    s_t[:], gloc[:, t:t + 1].broadcast_to([128, 128]),
                    iota[:], mybir.AluOpType.is_equal)
                nc.tensor.matmul(p_pool[:], s_t[:], h2[:],
                                 start=(t == 0), stop=(t == NB - 1))
                nc.tensor.matmul(p_cnt[:, 0:1], s_t[:], onecol[:],
                                 start=(t == 0), stop=(t == NB - 1))

            # ---- pooling combine + FC
            pool_sb = st.tile([128, H2 + 4], f32)
            nc.vector.memset(pool_sb[:, H2 + 1:], 0.0)
            nc.vector.tensor_copy(pool_sb[:, :H2], p_pool[:])
            nc.vector.tensor_copy(pool_sb[:, H2:H2 + 1], p_cnt[:, 0:1])
            nc.gpsimd.dma_start(pool_sh[:], pool_sb[:])
            nc.gpsimd.collective_compute(
                "AllGather", mybir.AluOpType.bypass,
                replica_groups=[list(range(NCORES))],
                ins=[pool_sh.opt()], outs=[pool_ag.opt()])

            agp = st.tile([128, NCORES, H2 + 4], f32)
            nc.sync.dma_start(
                agp[:], pool_ag[:].rearrange("(c p) f -> p c f", c=NCORES))
            meanT = st.tile([128, G], f32)
            GB = G // 128
            for gb in range(GB):
                pf = ps_big.tile([128, H2 + 4], f32, tag="big")
                for c in range(NCORES):
                    mt = tp.tile([128, 128], f32, tag="mt")
                    nc.sync.dma_start(
                        mt[:],
                        d_M.ap()[c * 128:(c + 1) * 128,
                                 gb * 128:(gb + 1) * 128])
                    nc.tensor.matmul(pf[:, :H2 + 1], mt[:],
                                     agp[:, c, :H2 + 1],
                                     start=(c == 0), stop=(c == NCORES - 1))
                cnt = tp.tile([128, 1], f32, tag="cnt")
                nc.vector.tensor_scalar_max(cnt[:], pf[:, H2:H2 + 1], 1.0)
                rec = tp.tile([128, 1], f32, tag="rec")
                nc.vector.reciprocal(rec[:], cnt[:])
                mean = tp.tile([128, H2], f32, tag="mean")
                nc.vector.tensor_tensor(mean[:], pf[:, :H2],
                                        rec[:].broadcast_to([128, H2]),
                                        mybir.AluOpType.mult)
                ptm = ps_big.tile([128, 128], f32, tag="big")
                nc.tensor.transpose(ptm[:], mean[:], ident[:])
                nc.vector.tensor_copy(meanT[:, gb * 128:(gb + 1) * 128],
                                      ptm[:])
            pfc = ps_big.tile([OUT_C, G], f32, tag="big")
            nc.tensor.matmul(pfc[:], wfc[:], meanT[:], start=True, stop=True)
            outsb = st.tile([OUT_C, G], f32)
            nc.vector.tensor_tensor(outsb[:], pfc[:],
                                    bfc[:].broadcast_to([OUT_C, G]),
                                    mybir.AluOpType.add)
            nc.sync.dma_start(d_out.ap(), outsb[:])

    nc.compile()
    return nc


_CACHE = {}


def _run(inputs, trace=False):
    from concourse.bass_utils import run_bass_kernel_spmd

    edge_index = np.asarray(inputs["edge_index"])
    batch = np.asarray(inputs["batch"])
    key = "k"
    if key not in _CACHE:
        meta, per_core = preprocess(edge_index, batch)
        nc = build_kernel(meta)
        _CACHE[key] = (meta, per_core, nc)
    meta, per_core, nc = _CACHE[key]

    x = np.asarray(inputs["x"], np.float32)
    W1 = np.asarray(inputs["W1"], np.float32)
    b1 = np.asarray(inputs["b1"], np.float32).reshape(1, H1)
    W2 = np.asarray(inputs["W2"], np.float32)
    b2 = np.asarray(inputs["b2"], np.float32).reshape(1, H2)
    Wfc = np.asarray(inputs["Wfc"], np.float32)
    bfc = np.asarray(inputs["bfc"], np.float32).reshape(OUT_C, 1)
    ident = np.eye(128, dtype=np.float32)

    in_maps = []
    for c in range(NCORES):
        pc = per_core[c]
        in_maps.append({
            "xt": _xt_img(x[c * SH:(c + 1) * SH]),
            "w1": W1, "b1r": b1, "w2": W2, "b2r": b2,
            "wfc": Wfc, "bfc2": bfc,
            "rp_lo": pc["rp_lo"], "rp_hi": pc["rp_hi"],
            "idx_img": pc["idx_img"], "slot_img": pc["slot_img"],
            "glocal_img": pc["glocal_img"],
            "Mmat": meta["M"], "ident": ident,
        })
    res = run_bass_kernel_spmd(nc, in_maps, list(range(NCORES)), trace=trace)
    out = res.results[0]["outT"].T.copy()  # [G, 2]
    return out.astype(np.float32), res


def kernel(**inputs):
    out, _ = _run(inputs)
    return out


# numpy simulation of the device algorithm (for validation)
def numpy_sim(inputs, meta, per_core, use_bf16=True):
    import ml_dtypes

    bf16 = ml_dtypes.bfloat16

    def q(a):
        return a.astype(bf16).astype(np.float32) if use_bf16 else a

    x = np.asarray(inputs["x"], np.float32)
    W1 = np.asarray(inputs["W1"], np.float32)
    b1 = np.asarray(inputs["b1"], np.float32)
    W2 = np.asarray(inputs["W2"], np.float32)
    b2 = np.asarray(inputs["b2"], np.float32)
    Wfc = np.asarray(inputs["Wfc"], np.float32)
    bfc = np.asarray(inputs["bfc"], np.float32)

    ntt = meta["ntt"]
    tile_blk = meta["tile_blk"]
    ntt_w = meta["ntt_w"]

    # per-core dinv
    dinv = []
    for c in range(NCORES):
        pc = per_core[c]
        deg = (pc["rp_hi"].astype(np.int64) - pc["rp_lo"].astype(np.int64)) + 1
        dinv.append(1.0 / np.sqrt(deg.astype(np.float32)))  # [128, NB]

    def seg_layer(z_full_q, core):
        """z_full_q: [8*SHP, TROW] quantized table; returns agg [128, NB, 64]."""
        pc = per_core[core]
        agg = np.zeros((128, NB, H1), np.float32)
        tbase = 0
        for w in range(NW):
            for ti in range(ntt_w[w]):
                t = tbase + ti
                b = int(tile_blk[w][ti])
                idxs = pc["idx_img"][:16, t * 8 : (t + 1) * 8].T.reshape(-1)  # 128
                rows = z_full_q[w * WIN + idxs.astype(np.int64), :H1]  # [128, 64]
                slots = pc["slot_img"][:, t].astype(np.int64)  # [128]
                S = np.zeros((128, 128), np.float32)
                val = slots >= 0
                S[np.arange(128)[val], slots[val]] = 1.0
                agg[:, b, :] += S.T @ rows
            tbase += ntt_w[w]
        return agg

    # layer 1 tables
    z1_full = np.zeros((NCORES * SHP, TROW), np.float32)
    xw1_all = []
    for c in range(NCORES):
        xt = _xt_img(x[c * SH : (c + 1) * SH])  # [3, SHP]
        xw1 = (xt.T @ W1)  # [SHP, 64]
        xw1_img = xw1.reshape(NB, 128, H1).transpose(1, 0, 2)  # [128, NB, 64]
        z1 = xw1_img * dinv[c][:, :, None]
        # table rows partition-major: row p*NB+t
        z1_full[c * SHP : (c + 1) * SHP, :H1] = q(z1).transpose(0, 1, 2).reshape(
            128 * NB, H1
        )
        xw1_all.append(xw1_img)
    z1q = q(z1_full)

    h1_all, z2_all = [], []
    z2_full = np.zeros((NCORES * SHP, TROW), np.float32)
    for c in range(NCORES):
        agg1 = seg_layer(z1q, c)
        z1_loc = z1q[c * SHP : (c + 1) * SHP, :H1].reshape(128, NB, H1)
        h1 = np.maximum(dinv[c][:, :, None] * (agg1 + z1_loc) + b1, 0.0)
        z2 = h1 * dinv[c][:, :, None]
        z2_full[c * SHP : (c + 1) * SHP, :H1] = q(z2).reshape(128 * NB, H1)
        h1_all.append(h1)
        z2_all.append(z2)
    z2q = q(z2_full)

    pool_part = np.zeros((NCORES, 128, H2 + 1), np.float32)
    for c in range(NCORES):
        agg2 = seg_layer(z2q, c)
        z2_loc = z2q[c * SHP : (c + 1) * SHP, :H1].reshape(128, NB, H1)
        pre2 = dinv[c][:, :, None] * (agg2 + z2_loc)
        h2 = np.maximum(pre2 @ W2 + b2, 0.0)  # [128, NB, 128]
        h2q = q(h2)
        pc = per_core[c]
        for t in range(NB):
            slots = pc["glocal_img"][:, t].astype(np.int64)
            Sp = np.zeros((128, 128), np.float32)
            val = slots >= 0
            Sp[np.arange(128)[val], slots[val]] = 1.0
            pool_part[c, :, :H2] += Sp.T @ h2q[:, t, :]
            pool_part[c, :, H2] += Sp.sum(axis=0)

    ag = pool_part.reshape(NCORES * 128, H2 + 1)
    full = meta["M"].T @ ag  # [512, 129]
    cnt = np.maximum(full[:, H2], 1.0)
    mean = full[:, :H2] / cnt[:, None]
    return mean @ Wfc + bfc
